# revision 16
# baseline (speedup 1.0000x reference)
"""MiniBatchKMeans partial_fit step on 8 Trainium2 NeuronCores (Bass/Tile).

Strategy (data-parallel over the batch, per sharding hint):
  - batch [65536,128] is split into 8 shards of 8192 rows, one per core.
  - Each core computes, for its shard, in 64 blocks of 128 rows:
      S~ = X.Ct - ||c||^2/2          (PE: rank-1 bias matmul + fp32r matmuls)
      negm = -max_k S~               (DVE reduce, negate)
      E = Sign(S~ - m)               (ACT; bf16; affine one-hot: max->0, else->-1)
      sumsT += X^T E                 (PE bf16 matmuls, PSUM accumulate)
      counts += 1^T E                (PE bf16 matmuls)
  - Host gathers per-core {sumsT, counts, negm}, undoes the Sign affine map
    (E_true = E + 1 => sums_true = sumsT + colsum(X), counts_true = counts + B),
    all-reduces over cores, and finishes the tiny [K,D]-sized center update,
    empty-cluster replacement, inertia, and collapsed-center split exactly as
    the reference does. The collapse split path falls back to an exact host
    computation only when collapsed centers exist (never for gaussian data).

The Sign one-hot can double-count a row if two centers tie bitwise in fp32
at the row max; the affine correction stays exact for counts either way.
fp32r matmuls carry ~1.6e-4 relative error, which can flip the argmin for
near-equidistant points; the end-to-end error stays ~1e-2-scale-relative.

Hardware quirks encoded here (measured on this runtime):
  - fused 4-byte (fp32/f32r) matmuls/transposes accept only ONE sync wait:
    dependencies are funneled through a single engine, or absorbed by a tiny
    bf16 dummy matmul issued right before.
  - native TENSOR_TENSOR_REDUCE crashes the exec unit: bias is folded on PE.
  - f32r operands must be produced by a compute op (DVE copy), not plain DMA.
"""

import numpy as np

B_FULL = 65536
D = 128
K = 1024
N_CORES = 8
P = 128

_COMPILED = {}


def _build(nc_mod, B_shard, sign_zero_is_one=False):
    """Build the per-core Bass program for a B_shard-row shard."""
    import concourse.bacc as bacc
    import concourse.mybir as mybir
    import concourse.tile as tile

    F32 = mybir.dt.float32
    F32R = mybir.dt.float32r
    BF16 = mybir.dt.bfloat16
    AT = mybir.ActivationFunctionType
    ALU = mybir.AluOpType

    n_blocks = B_shard // P
    SLAB = 8 if n_blocks % 8 == 0 else 1   # blocks per DMA slab
    n_slabs = n_blocks // SLAB

    nc = bacc.Bacc("TRN2", target_bir_lowering=False, debug=False,
                   num_devices=N_CORES)

    x_d = nc.dram_tensor("x", [B_shard, D], F32, kind="ExternalInput")
    c_d = nc.dram_tensor("c", [K, D], F32, kind="ExternalInput")
    negh_d = nc.dram_tensor("negh", [1, K], F32, kind="ExternalInput")

    sums_d = nc.dram_tensor("sumsT", [D, K], F32, kind="ExternalOutput")
    counts_d = nc.dram_tensor("counts", [1, K], F32, kind="ExternalOutput")
    negm_d = nc.dram_tensor("negm", [P, n_blocks], F32, kind="ExternalOutput")

    with tile.TileContext(nc) as tc:
        with (
            tc.tile_pool(name="const", bufs=1) as cpool,
            tc.tile_pool(name="slab", bufs=2) as slab_pool,
            tc.tile_pool(name="work", bufs=2) as work,
            tc.tile_pool(name="ps_s", bufs=2, space="PSUM") as ps_s,
            tc.tile_pool(name="ps_t", bufs=1, space="PSUM") as ps_t,
            tc.tile_pool(name="ps_acc", bufs=1, space="PSUM") as ps_acc,
        ):
            # ---------------- preamble (one-time) ----------------
            ident = cpool.tile([P, P], F32, tag="ident")
            from concourse.masks import make_identity
            make_identity(nc, ident[:])

            # centers [K, D] -> SBUF as 8 chunks of [128 rows, D]
            c_sb = cpool.tile([P, 8, D], F32, tag="c_sb")
            nc.sync.dma_start(
                c_sb[:], c_d[:].rearrange("(a p) d -> p a d", p=P))

            # negh [1, K] split into bf16 hi + lo rows of one [2, K] tile; the
            # bias is then a single contraction-2 matmul per half
            # (ones2^T @ [hi; lo] = hi + lo broadcast over rows)
            negh_sb = cpool.tile([1, K], F32, tag="negh_sb")
            nc.sync.dma_start(negh_sb[:], negh_d[:])
            negh_pair = cpool.tile([2, K], BF16, tag="negh_pair")
            nc.scalar.activation(negh_pair[0:1, :], negh_sb[:], AT.Copy)
            negh_lo = cpool.tile([1, K], BF16, tag="negh_lo")
            nc.vector.tensor_tensor(
                out=negh_lo[:], in0=negh_sb[:], in1=negh_pair[0:1, :],
                op=mybir.AluOpType.subtract)
            # engines cannot write at partition offset 1; place row 1 via DMA
            nc.sync.dma_start(negh_pair[1:2, :], negh_lo[:])

            # ones for the bias rank-2 matmul and counts
            ones_row = cpool.tile([2, P], BF16, tag="ones_row")
            nc.vector.memset(ones_row[:], 1.0)
            ones_col = cpool.tile([P, 1], BF16, tag="ones_col")
            nc.vector.memset(ones_col[:], 1.0)

            # CT = C^T [D, K]: 8 PE transposes; split into bf16 hi + lo so the
            # distance matmul runs as exact bf16 products (hi*hi + hi*lo +
            # lo*hi), avoiding fp32r's ~2^-13 operand rounding
            ct_f = cpool.tile([P, K], F32, tag="ct_f")
            ct_hi = cpool.tile([P, K], BF16, tag="ct_hi")
            ct_lo = cpool.tile([P, K], BF16, tag="ct_lo")
            tr_ps = ps_t.tile([P, P], F32, tag="tr")
            for a in range(8):
                # dummy bf16 mm absorbs {gpsimd identity / DMA / DVE} waits so
                # the fused fp32 transpose keeps a single wait
                nc.tensor.matmul(
                    tr_ps[0:1, 0:1], ident[:].bitcast(BF16)[:, 1:2],
                    c_sb[:, a, :].bitcast(BF16)[:, 1:2], start=True, stop=True)
                nc.tensor.transpose(tr_ps[:], c_sb[:, a, :], ident[:])
                nc.vector.tensor_copy(ct_f[:, a * P:(a + 1) * P], tr_ps[:])
            nc.scalar.activation(ct_hi[:], ct_f[:], AT.Copy)
            nc.vector.tensor_tensor(
                out=ct_lo[:], in0=ct_f[:], in1=ct_hi[:],
                op=mybir.AluOpType.subtract)

            # negm accumulator [P, n_blocks]
            negm_sb = cpool.tile([P, n_blocks], F32, tag="negm_sb")

            # persistent PSUM accumulators. counts halves live in one bank at
            # partitions 0 and 32 (PSUM budget: 4+1+2+1 = 8 banks exactly).
            sums_ps = ps_acc.tile([P, K], F32, tag="sums_ps")       # 2 banks
            counts_ps = ps_acc.tile([P, 512], F32, tag="counts_ps")  # 1 bank

            # ---------------- main loop ----------------
            for b in range(n_blocks):
                s, j = divmod(b, SLAB)
                if j == 0:
                    x_slab = slab_pool.tile([P, SLAB, D], F32, tag="x_slab")
                    nc.sync.dma_start(
                        x_slab[:],
                        x_d[s * SLAB * P:(s + 1) * SLAB * P, :]
                        .rearrange("(a p) d -> p a d", p=P))
                x_blk = x_slab[:, j, :]

                # X block -> bf16 (scatter lhsT) on ACT
                x_bf = work.tile([P, D], BF16, tag="x_bf")
                nc.scalar.activation(x_bf[:], x_blk, AT.Copy)

                # transpose X block; dummy bf16 mm absorbs {DMA, DVE} waits
                nc.tensor.matmul(
                    tr_ps[0:1, 0:1], ident[:].bitcast(BF16)[:, 1:2],
                    x_blk.bitcast(BF16)[:, 1:2], start=True, stop=True)
                nc.tensor.transpose(tr_ps[:], x_blk, ident[:])
                xt_f = work.tile([P, P], F32, tag="xt_f")
                nc.vector.tensor_copy(xt_f[:], tr_ps[:])
                xt_hi = work.tile([P, P], BF16, tag="xt_hi")
                nc.scalar.activation(xt_hi[:], xt_f[:], AT.Copy)
                xt_lo = work.tile([P, P], BF16, tag="xt_lo")
                nc.vector.tensor_tensor(
                    out=xt_lo[:], in0=xt_f[:], in1=xt_hi[:],
                    op=mybir.AluOpType.subtract)

                # s~ = -h (2 rank-1 bias mms, exact bf16 pair) + X.Ct via
                # 3-term bf16 split: hi*hi + hi*lo + lo*hi (error ~1e-5)
                s_ps = ps_s.tile([P, K], F32, tag="s_ps")
                for half in range(2):
                    cols = slice(half * 512, (half + 1) * 512)
                    nc.tensor.matmul(s_ps[:, cols], ones_row[:],
                                     negh_pair[:, cols], start=True, stop=False,
                                     skip_group_check=True)
                for half in range(2):
                    cols = slice(half * 512, (half + 1) * 512)
                    nc.tensor.matmul(s_ps[:, cols], xt_hi[:],
                                     ct_hi[:, cols], start=False, stop=False,
                                     skip_group_check=True)
                    nc.tensor.matmul(s_ps[:, cols], xt_hi[:],
                                     ct_lo[:, cols], start=False, stop=False,
                                     skip_group_check=True)
                    nc.tensor.matmul(s_ps[:, cols], xt_lo[:],
                                     ct_hi[:, cols], start=False, stop=True,
                                     skip_group_check=True)

                # negm = -max_k s~
                negm_col = negm_sb[:, b:b + 1]
                nc.vector.tensor_reduce(
                    negm_col, s_ps[:], axis=mybir.AxisListType.X,
                    op=ALU.max, negate=True)

                # E = Sign(s~ - m): row max -> 0 (or +1), others -> -1
                e_bf = work.tile([P, K], BF16, tag="e_bf")
                nc.scalar.activation(e_bf[:], s_ps[:], AT.Sign,
                                     bias=negm_col, scale=1.0)

                # sumsT += X^T E ; counts += 1^T E
                first, last = (b == 0), (b == n_blocks - 1)
                for half in range(2):
                    cols = slice(half * 512, (half + 1) * 512)
                    nc.tensor.matmul(sums_ps[:, cols], x_bf[:], e_bf[:, cols],
                                     start=first, stop=last,
                                     skip_group_check=True)
                for half in range(2):
                    cols = slice(half * 512, (half + 1) * 512)
                    nc.tensor.matmul(
                        counts_ps[32 * half:32 * half + 1, :], ones_col[:],
                        e_bf[:, cols], start=first, stop=last,
                        skip_group_check=True)

            # ---------------- epilogue ----------------
            sums_sb = cpool.tile([P, K], F32, tag="sums_sb")
            nc.vector.tensor_copy(sums_sb[:], sums_ps[:])
            nc.sync.dma_start(sums_d[:], sums_sb[:])
            counts_sb = cpool.tile([1, K], F32, tag="counts_sb")
            nc.vector.tensor_copy(counts_sb[:, 0:512], counts_ps[0:1, :])
            nc.vector.tensor_copy(counts_sb[:, 512:1024], counts_ps[32:33, :])
            nc.sync.dma_start(counts_d[:], counts_sb[:])
            nc.sync.dma_start(negm_d[:], negm_sb[:])

    nc.compile()
    return nc


def _get_compiled(B_shard):
    key = B_shard
    if key not in _COMPILED:
        _COMPILED[key] = _build(None, B_shard)
    return _COMPILED[key]


LAST_EXEC_NS = None


def _run_device(batch, centers, n_cores=N_CORES, sim=False, trace=False):
    """Run the SPMD kernel; returns per-core lists (sumsT, counts, negm)."""
    global LAST_EXEC_NS
    B, D_ = batch.shape
    B_shard = B // n_cores
    nc = _get_compiled(B_shard)

    negh = -0.5 * (centers.astype(np.float64) ** 2).sum(axis=1)
    negh = negh.astype(np.float32).reshape(1, K)

    in_maps = []
    for c in range(n_cores):
        in_maps.append({
            "x": np.ascontiguousarray(batch[c * B_shard:(c + 1) * B_shard]),
            "c": centers,
            "negh": negh,
        })

    if sim:
        from concourse.bass_interp import CoreSim
        results = []
        for m in in_maps:
            s = CoreSim(nc)
            for k_, v_ in m.items():
                s.tensor(k_)[:] = v_
            s.simulate()
            results.append({k_: np.array(s.tensor(k_))
                            for k_ in ("sumsT", "counts", "negm")})
        return results

    from concourse.bass_utils import run_bass_kernel_spmd
    if trace:
        try:
            out = run_bass_kernel_spmd(
                nc, in_maps, core_ids=list(range(n_cores)), trace=True)
            LAST_EXEC_NS = out.exec_time_ns
            return out.results
        except ModuleNotFoundError:
            pass  # no axon NTFF hook in this container
    out = run_bass_kernel_spmd(nc, in_maps, core_ids=list(range(n_cores)))
    LAST_EXEC_NS = out.exec_time_ns
    return out.results


def _cdist_np(a, b):
    a2 = (a * a).sum(-1)[:, None]
    b2 = (b * b).sum(-1)[None, :]
    d2 = np.maximum(a2 + b2 - 2.0 * (a @ b.T), 0.0)
    return np.sqrt(d2)


COLLAPSE_TOL = 0.5


def kernel(batch, centers, counts, _sim=False, _n_cores=N_CORES, _trace=False):
    batch = np.asarray(batch, dtype=np.float32)
    centers = np.asarray(centers, dtype=np.float32)
    counts = np.asarray(counts, dtype=np.float32)
    B, D_ = batch.shape
    K_, _ = centers.shape

    results = _run_device(batch, centers, n_cores=_n_cores, sim=_sim,
                          trace=_trace)

    # --- undo the Sign affine map and all-reduce over cores ---
    # sign(s-m): max -> sign(0) = z0 (0 on sim; calibrated at runtime),
    # others -> -1. E_true = (E - z0*?) handled for both conventions:
    # if z0 == 0:  E_true = E + 1
    # if z0 == 1:  E_true = (E + 1)/2
    sumsT = np.zeros((D, K_), dtype=np.float64)
    counts_dev = np.zeros((K_,), dtype=np.float64)
    negm_all = []
    for r in results:
        sumsT += r["sumsT"].astype(np.float64)
        counts_dev += r["counts"].reshape(-1).astype(np.float64)
        negm_all.append(r["negm"].astype(np.float32))

    # The device scatter uses X rounded to bf16 (ACT Copy convert); the
    # affine-map correction must add the colsum of the SAME rounded values.
    import ml_dtypes
    x_bf = batch.astype(ml_dtypes.bfloat16).astype(np.float64)
    colsum = x_bf.sum(axis=0)                              # [D]
    # counts_true(+1 map) = counts_dev + B ; sum must equal B (mod rare ties)
    # counts_true(half map) = (counts_dev + B)/2
    c1 = counts_dev + B
    if abs(c1.sum() - B) <= abs(c1.sum() / 2.0 - B):
        counts_b = c1
        sums_b = sumsT + colsum[:, None]
    else:
        counts_b = c1 / 2.0
        sums_b = (sumsT + colsum[:, None]) / 2.0

    counts_batch = counts_b.astype(np.float32)             # [K]
    sums_batch = sums_b.T.astype(np.float32)               # [K, D]

    # --- min distance per row (for inertia): d2 = ||x||^2 + 2*negm ---
    negm = np.concatenate([m.T.reshape(-1) for m in negm_all])  # [B] row-major
    # negm layout per core: [P, n_blocks], col b holds rows [b*128, (b+1)*128)
    x2 = (batch.astype(np.float64) ** 2).sum(axis=1)
    d2min = x2 + 2.0 * negm.astype(np.float64)
    d2min = np.maximum(d2min, 0.0)
    inertia = np.float32(d2min.mean())

    # --- empty-cluster replacement (deterministic jax key 42) ---
    import jax
    repl_idx = np.asarray(
        jax.random.randint(jax.random.key(42), (K_,), 0, B))
    replacement = batch[repl_idx]                          # [K, D]
    empty = (counts == 0) & (counts_batch == 0)
    counts_batch = np.where(empty, np.float32(1.0), counts_batch)
    sums_batch = np.where(empty[:, None], replacement, sums_batch)

    new_counts = counts + counts_batch
    mask = counts_batch > 0
    updated = (centers * counts[:, None] + sums_batch) / new_counts[:, None]
    new_centers = np.where(mask[:, None], updated, centers).astype(np.float32)

    # --- split collapsed centers ---
    pdist = _cdist_np(new_centers, new_centers)
    close = np.triu(pdist < COLLAPSE_TOL, k=1)
    cnt_le = new_counts[:, None] <= new_counts[None, :]
    loser_i = np.any(close & cnt_le, axis=1)
    loser_j = np.any(close & ~cnt_le, axis=0)
    loser = loser_i | loser_j

    if loser.any():
        # exact (slow) host fallback; unreachable for gaussian-random data
        cand_dist = _cdist_np(batch, new_centers)
        farthest = cand_dist.max(axis=1)
        order = np.argsort(-farthest, kind="stable")
        rank = np.cumsum(loser.astype(np.int32)) - 1
        repl2 = batch[order[np.clip(rank, 0, B - 1)]]
        new_centers = np.where(loser[:, None], repl2, new_centers)
        new_counts = np.where(loser, np.float32(0.0), new_counts)

    return (new_centers.astype(np.float32),
            new_counts.astype(np.float32),
            inertia)
